# revision 26
# baseline (speedup 1.0000x reference)
"""Multi-head attention (B=2, S=2048, E=1024, H=16, d_h=64, causal) on 8
Trainium2 NeuronCores, bf16 matmuls (rel err ~5e-3, gate 2e-2).

Sharding: tensor-parallel over heads (2 heads/core) for QKV projections and
attention; AllToAll of the concatenated head outputs (1MB/core bf16); then
sequence-parallel output projection (each core computes 256 output rows per
batch).

v2 changes vs v1:
  - x is transposed + cast to bf16 on the host -> x^T tiles DMA straight
    into SBUF (HWDGE on sync), eliminating all PE transposes of x and their
    PSUM evictions, and the SWDGE cast bottleneck at kernel start.
  - all weights host-cast to bf16, HWDGE loads.
  - all 16 x^T tiles (both batches) prefetched at kernel start.
  - cg gathers moved to the sync queue so a slow collective cannot block
    the gpsimd queue (which attention's normalize needs) -> no cross-batch
    stall cascade.
  - normalize reads the AV accumulator straight from PSUM (drops the avsb
    eviction copy on ACT).
  - vst tiles keep a persistent ones-column (written once per slot).

Layouts (all "transposed", partition = contraction dim):
  x^T [e, s] from host; Q^T/K^T/V^T per head [64 d, 2048 s] from projection
  scores S^T [t, q] (stationary = K^T slice, moving = Q^T)
  exp on ACT; causal diagonal handled by adding a -8000 triangle pre-exp
  AV with stationary [V_chunk | ones] [128, 65] -> psum row 64 = softmax
  denominator; ACT shifts it to partition 0, DVE reciprocal, gpsimd
  partition_broadcast, DVE multiply normalizes into C^T
  W_O with stationary C^T chunks -> output directly in [q, e] layout.

Hardware constraints honored (empirical): matmul operands share
base_partition 0; DVE cannot shift partitions (ACT can); PSUM has no DMA
route.
"""

import numpy as np
from ml_dtypes import bfloat16

import concourse.bacc as bacc
import concourse.mybir as mybir
import concourse.tile as tile
from concourse.bass_utils import run_bass_kernel_spmd
from concourse.masks import make_identity

F32 = mybir.dt.float32
BF16 = mybir.dt.bfloat16
AF = mybir.ActivationFunctionType

N_CORES = 8
B, S, E = 2, 2048, 1024
H, DH = 16, 64
HPC = H // N_CORES  # heads per core = 2
QS = S // N_CORES  # output q rows per core per batch = 256
SCALE = 1.0 / 8.0  # 1/sqrt(DH)

_NC_CACHE = []


def build_nc():
    nc = bacc.Bacc("TRN2", target_bir_lowering=False, debug=False, num_devices=N_CORES)

    xt_d = nc.dram_tensor("xt", [B, E, S], BF16, kind="ExternalInput").ap()
    wq_d = nc.dram_tensor("wq", [E, HPC * DH], BF16, kind="ExternalInput").ap()
    wk_d = nc.dram_tensor("wk", [E, HPC * DH], BF16, kind="ExternalInput").ap()
    wv_d = nc.dram_tensor("wv", [E, HPC * DH], BF16, kind="ExternalInput").ap()
    wo_d = nc.dram_tensor("wo", [E, E], BF16, kind="ExternalInput").ap()
    out_d = nc.dram_tensor("out", [B, QS, E], F32, kind="ExternalOutput").ap()

    with tile.TileContext(nc, trace_sim=False) as tc:
        with (
            tc.tile_pool(name="const", bufs=1) as constp,
            tc.tile_pool(name="wpool", bufs=1) as wpool,
            tc.tile_pool(name="wop", bufs=1) as wop,
            tc.tile_pool(name="xep", bufs=2) as xep,
            tc.tile_pool(name="qkv", bufs=2) as qkvp,
            tc.tile_pool(name="vst", bufs=2) as vstp,
            tc.tile_pool(name="pt", bufs=3) as ptp,
            tc.tile_pool(name="ct", bufs=2) as ctp,
            tc.tile_pool(name="norm", bufs=2) as normp,
            tc.tile_pool(name="cg", bufs=1) as cgp,
            tc.tile_pool(name="osb", bufs=2) as osbp,
            tc.tile_pool(name="psb", bufs=2, space="PSUM") as psb,  # [128,1024] x2 = 4 banks
            tc.tile_pool(name="psm", bufs=2, space="PSUM") as psm,  # [128,512] x2 = 2 banks
            tc.tile_pool(name="pav", bufs=2, space="PSUM") as pav,  # [65,512] x2 = 2 banks
            tc.tile_pool(name="dram", bufs=2, space="DRAM") as dramp,
        ):
            identb = constp.tile([128, 128], BF16, tag="identb")
            make_identity(nc, identb[:])
            ones_col = constp.tile([128, 1], BF16, tag="ones_col")
            nc.gpsimd.memset(ones_col[:], 1.0)
            # causal diagonal mask: 0 where q_rel >= t_rel else -8000
            # (accumulated into the diagonal score block BY THE PE, via
            # identity.T @ mtri -- keeps every other engine queue free of
            # mask dependencies; -8000 is exact in bf16)
            mtri = constp.tile([128, 128], BF16, tag="mtri")
            nc.gpsimd.memset(mtri[:], 0.0)
            nc.gpsimd.affine_select(
                out=mtri[:], in_=mtri[:],
                compare_op=mybir.AluOpType.is_ge, fill=-8000.0,
                base=0, pattern=[[1, 128]], channel_multiplier=-1,
            )

            # host pre-packs weights as [p, chunk, col] so each tensor is one
            # contiguous DMA; stationary tiles are slices of the big tile.
            # wq first, then batch-0 x^T chunks, so the first projection's
            # inputs land as early as possible.
            wall = {}
            wall["q"] = wpool.tile([128, 8 * 128], BF16, tag="wq_all", name="wq_all")
            nc.sync.dma_start(
                out=wall["q"][:].rearrange("p (c d) -> p c d", c=8),
                in_=wq_d[:].rearrange("(c p) d -> p c d", p=128),
            )

            # x^T tiles for both batches, prefetched up front on the sync
            # HWDGE queue: xe[b][ec] = x^T[e-chunk ec, all 2048 q] bf16
            xe = {}
            for ec in range(8):
                t = xep.tile([128, S], BF16, tag=f"xe{ec}", name=f"xe0_{ec}")
                nc.sync.dma_start(out=t[:], in_=xt_d[0, ec * 128:(ec + 1) * 128, :])
                xe[0, ec] = t

            for name, wd in (("k", wk_d), ("v", wv_d)):
                wall[name] = wpool.tile([128, 8 * 128], BF16, tag=f"w{name}_all", name=f"w{name}_all")
                nc.sync.dma_start(
                    out=wall[name][:].rearrange("p (c d) -> p c d", c=8),
                    in_=wd[:].rearrange("(c p) d -> p c d", p=128),
                )

            def wtiles(name, ec):
                return wall[name][:, ec * 128:(ec + 1) * 128]

            wo_all = wop.tile([128, 8 * 1024], BF16, tag="wo_all")
            nc.sync.dma_start(
                out=wo_all[:].rearrange("p (c d) -> p c d", c=8),
                in_=wo_d[:].rearrange("(c p) d -> p c d", p=128),
            )

            for ec in range(8):
                t = xep.tile([128, S], BF16, tag=f"xe{ec}", name=f"xe1_{ec}")
                nc.sync.dma_start(out=t[:], in_=xt_d[1, ec * 128:(ec + 1) * 128, :])
                xe[1, ec] = t

            copy_flip = [0]

            def copy_balanced(dst, src):
                # alternate psum->sbuf evictions between DVE and ACT
                if copy_flip[0] % 2 == 0:
                    nc.vector.tensor_copy(dst, src)
                else:
                    nc.scalar.copy(dst, src)
                copy_flip[0] += 1

            vst_init = {}

            def emit_batch(b):
                # ---- Phase A/B: QKV projections ---------------------------
                qkv = {}
                for name in ("q", "k", "v"):
                    for h in range(HPC):
                        qkv[name, h] = qkvp.tile([64, S], BF16, tag=f"{name}h{h}", name=f"{name}h{h}")
                for qg in range(4):
                    sl = slice(qg * 512, qg * 512 + 512)
                    for name in ("q", "k", "v"):
                        ps = psm.tile([128, 512], F32, tag="mm")
                        for ec in range(8):
                            nc.tensor.matmul(
                                ps[:],
                                wtiles(name, ec),
                                xe[b, ec][:, sl],
                                start=(ec == 0),
                                stop=(ec == 7),
                            )
                        nc.vector.tensor_copy(qkv[name, 0][:, sl], ps[0:64, :])
                        nc.scalar.copy(qkv[name, 1][:, sl], ps[64:128, :])

                # ---- Phase C: attention per head ---------------------------
                ct = [ctp.tile([64, S], BF16, tag=f"ct{h}", name=f"ct{h}") for h in range(HPC)]
                # one AllToAll per batch: shard j carries BOTH heads' C^T
                # columns for core j ([128, QS] stacked) -> half the sync
                # points vs per-head collectives
                a2a_in = dramp.tile([8, 2 * DH, QS], BF16, tag="a2a_in", name=f"a2a_in{b}")
                a2a_out = dramp.tile([8, 2 * DH, QS], BF16, tag="a2a_out", name=f"a2a_out{b}")
                for h in range(HPC):
                    vh = qkv["v", h]
                    # Vst[tc]: [128 t, 65] = [V_chunk | ones]
                    vst = []
                    for tg in range(2):  # pack 8 transposes per psum tile
                        ps = psm.tile([128, 512], BF16, tag="mm")
                        for k in range(8):
                            tcx = tg * 8 + k
                            nc.tensor.transpose(
                                ps[:, k * 64:(k + 1) * 64],
                                vh[:, tcx * 128:(tcx + 1) * 128],
                                identb[0:64, 0:64],
                            )
                        for k in range(8):
                            tcx = tg * 8 + k
                            vt = vstp.tile(
                                [128, 65], BF16, tag=f"vst{tcx}",
                                name=f"vst{tcx}",
                            )
                            nc.vector.tensor_copy(vt[:, 0:64], ps[:, k * 64:(k + 1) * 64])
                            # ones column persists in the slot; write it only
                            # the first time each slot comes around
                            cnt = vst_init.get(tcx, 0)
                            if cnt < 2:
                                nc.vector.tensor_copy(vt[:, 64:65], ones_col[:])
                                vst_init[tcx] = cnt + 1
                            vst.append(vt)

                    kh, qh = qkv["k", h], qkv["q", h]
                    for half in range(2):
                        qbase = half * 1024
                        n_tc = 8 * (half + 1)
                        av = [
                            pav.tile([65, 512], F32, tag="av", name=f"av{i}")
                            for i in range(2)
                        ]
                        for tcx in range(n_tc):
                            t0 = tcx * 128
                            q_lo = max(t0, qbase)
                            lo_rel = q_lo - qbase
                            strip = psb.tile([128, 1024], F32, tag="big")
                            # scores into strip (columns relative to qbase);
                            # ISA caps matmul output at 512 fp32 cols. The
                            # causal diagonal block (always inside the first
                            # seg) gets -8000 accumulated by a second PE
                            # matmul (identity stationary, mtri moving).
                            diag = t0 >= qbase
                            segs = []
                            if lo_rel < 512:
                                segs.append((lo_rel, 512))
                                segs.append((512, 1024))
                            else:
                                segs.append((lo_rel, 1024))
                            for s0, s1 in segs:
                                first = s0 == lo_rel
                                nc.tensor.matmul(
                                    strip[:, s0:s1],
                                    kh[:, t0:t0 + 128],
                                    qh[:, qbase + s0:qbase + s1],
                                    start=True,
                                    stop=not (diag and first),
                                )
                                if diag and first:
                                    nc.tensor.matmul(
                                        strip[:, lo_rel:lo_rel + 128],
                                        identb[:],
                                        mtri[:],
                                        start=False,
                                        stop=True,
                                    )
                            pt = ptp.tile([128, 1024], BF16, tag="pt")
                            nc.scalar.activation(
                                pt[:, lo_rel:1024],
                                strip[:, lo_rel:1024],
                                AF.Exp,
                                scale=SCALE,
                            )
                            for qbr in range(2):
                                qb = 2 * half + qbr
                                if qb * 512 + 512 <= t0:
                                    continue
                                m_lo = max(t0, qb * 512)
                                nc.tensor.matmul(
                                    av[qbr][:, m_lo - qb * 512:512],
                                    vst[tcx][:],
                                    pt[:, m_lo - qbase:qb * 512 + 512 - qbase],
                                    start=(tcx == 0),
                                    stop=(tcx == (qb + 1) * 4 - 1),
                                )
                        # normalize the two q-blocks of this half
                        for qbr in range(2):
                            qb = 2 * half + qbr
                            # denominator row to partition 0 (ACT can shift)
                            nsb = normp.tile([1, 512], F32, tag="nsb")
                            nc.scalar.copy(nsb[:], av[qbr][64:65, :])
                            recip1 = normp.tile([1, 512], F32, tag="recip1")
                            nc.vector.reciprocal_approx_fast(recip1[:], nsb[:])
                            bc = normp.tile([64, 512], F32, tag="bc")
                            nc.gpsimd.partition_broadcast(bc[:], recip1[:])
                            nc.vector.tensor_mul(
                                ct[h][:, qb * 512:qb * 512 + 512],
                                av[qbr][0:64, :],
                                bc[:],
                            )
                            # ship this q-block's two a2a shards immediately
                            for j in (2 * qb, 2 * qb + 1):
                                nc.gpsimd.dma_start(
                                    out=a2a_in[j, h * DH:(h + 1) * DH, :],
                                    in_=ct[h][:, j * QS:(j + 1) * QS],
                                )

                nc.gpsimd.collective_compute(
                    "AllToAll",
                    mybir.AluOpType.bypass,
                    replica_groups=[list(range(N_CORES))],
                    ins=[a2a_in[:].opt()],
                    outs=[a2a_out[:].opt()],
                )

                # cg gather on the sync HWDGE queue (not gpsimd): a slow
                # collective then cannot block the gpsimd ops the next
                # batch's attention needs. One strided DMA per batch.
                cgbig = cgp.tile([128, 8 * QS], BF16, tag="cgbig", name=f"cg{b}", bufs=2)
                nc.sync.dma_start(
                    out=cgbig[:].rearrange("p (i q) -> p i q", i=8),
                    in_=a2a_out[:].rearrange("i p q -> p i q"),
                )
                return cgbig

            def emit_e(b, cgbig):
                # ---- Phase E: output projection ---------------------------
                for qt in range(QS // 128):
                    osb = osbp.tile([128, 1024], F32, tag="osb")
                    for oh in range(2):
                        ps = psm.tile([128, 512], F32, tag="mm")
                        for i in range(8):
                            nc.tensor.matmul(
                                ps[:],
                                cgbig[:, i * QS + qt * 128:i * QS + (qt + 1) * 128],
                                wo_all[:, i * 1024 + oh * 512:i * 1024 + (oh + 1) * 512],
                                start=(i == 0),
                                stop=(i == 7),
                            )
                        copy_balanced(osb[:, oh * 512:(oh + 1) * 512], ps[:])
                    nc.sync.dma_start(
                        out=out_d[b, qt * 128:(qt + 1) * 128, :], in_=osb[:]
                    )

            # Both E phases are emitted LAST with far-late scheduler
            # priority: without this the Tile scheduler hoists E(0) (which
            # depends on batch 0's AllToAll, a ~30us mesh-latency op) ahead
            # of batch 1's projections in the in-order PE queue, stalling
            # the whole machine. Late priority makes E(0) fill the final
            # collective's flight time instead.
            cgs = [emit_batch(b) for b in range(B)]
            with tc.high_priority(offset=-1000000):
                for b in range(B):
                    emit_e(b, cgs[b])

    nc.compile()
    return nc


def _get_nc():
    if not _NC_CACHE:
        _NC_CACHE.append(build_nc())
    return _NC_CACHE[0]


def run(inputs, trace=False, trace_cores=None):
    nc = _get_nc()
    x = np.asarray(inputs["x"], np.float32)
    xt = np.ascontiguousarray(x.transpose(0, 2, 1)).astype(bfloat16)
    Wq = np.asarray(inputs["Wq"], np.float32)
    Wk = np.asarray(inputs["Wk"], np.float32)
    Wv = np.asarray(inputs["Wv"], np.float32)
    W_O = np.ascontiguousarray(np.asarray(inputs["W_O"], np.float32)).astype(bfloat16)

    in_maps = []
    for j in range(N_CORES):
        h0 = HPC * j
        in_maps.append(
            {
                "xt": xt,
                "wq": np.ascontiguousarray(
                    np.concatenate([Wq[h0 + i] for i in range(HPC)], axis=1)
                ).astype(bfloat16),
                "wk": np.ascontiguousarray(
                    np.concatenate([Wk[h0 + i] for i in range(HPC)], axis=1)
                ).astype(bfloat16),
                "wv": np.ascontiguousarray(
                    np.concatenate([Wv[h0 + i] for i in range(HPC)], axis=1)
                ).astype(bfloat16),
                "wo": W_O,
            }
        )
    kwargs = {}
    if trace:
        kwargs["trace"] = True
        if trace_cores is not None:
            kwargs["trace_cores"] = trace_cores
    res = run_bass_kernel_spmd(nc, in_maps, core_ids=list(range(N_CORES)), **kwargs)
    out = np.empty((B, S, E), np.float32)
    for j in range(N_CORES):
        out[:, j * QS:(j + 1) * QS, :] = res.results[j]["out"]
    return out, res


def kernel(**inputs) -> np.ndarray:
    out, _ = run(inputs)
    return out


# revision 27
# speedup vs baseline: 1.0656x; 1.0656x over previous
"""Multi-head attention (B=2, S=2048, E=1024, H=16, d_h=64, causal) on 8
Trainium2 NeuronCores, bf16 matmuls (rel err ~5e-3, gate 2e-2).

Sharding: tensor-parallel over heads (2 heads/core) for QKV projections and
attention; AllToAll of the concatenated head outputs (1MB/core bf16); then
sequence-parallel output projection (each core computes 256 output rows per
batch).

v2 changes vs v1:
  - x is transposed + cast to bf16 on the host -> x^T tiles DMA straight
    into SBUF (HWDGE on sync), eliminating all PE transposes of x and their
    PSUM evictions, and the SWDGE cast bottleneck at kernel start.
  - all weights host-cast to bf16, HWDGE loads.
  - all 16 x^T tiles (both batches) prefetched at kernel start.
  - cg gathers moved to the sync queue so a slow collective cannot block
    the gpsimd queue (which attention's normalize needs) -> no cross-batch
    stall cascade.
  - normalize reads the AV accumulator straight from PSUM (drops the avsb
    eviction copy on ACT).
  - vst tiles keep a persistent ones-column (written once per slot).

Layouts (all "transposed", partition = contraction dim):
  x^T [e, s] from host; Q^T/K^T/V^T per head [64 d, 2048 s] from projection
  scores S^T [t, q] (stationary = K^T slice, moving = Q^T)
  exp on ACT; causal diagonal handled by adding a -8000 triangle pre-exp
  AV with stationary [V_chunk | ones] [128, 65] -> psum row 64 = softmax
  denominator; ACT shifts it to partition 0, DVE reciprocal, gpsimd
  partition_broadcast, DVE multiply normalizes into C^T
  W_O with stationary C^T chunks -> output directly in [q, e] layout.

Hardware constraints honored (empirical): matmul operands share
base_partition 0; DVE cannot shift partitions (ACT can); PSUM has no DMA
route.
"""

import numpy as np
from ml_dtypes import bfloat16

import concourse.bacc as bacc
import concourse.mybir as mybir
import concourse.tile as tile
from concourse.bass_utils import run_bass_kernel_spmd
from concourse.masks import make_identity

F32 = mybir.dt.float32
BF16 = mybir.dt.bfloat16
AF = mybir.ActivationFunctionType

N_CORES = 8
B, S, E = 2, 2048, 1024
H, DH = 16, 64
HPC = H // N_CORES  # heads per core = 2
QS = S // N_CORES  # output q rows per core per batch = 256
SCALE = 1.0 / 8.0  # 1/sqrt(DH)

_NC_CACHE = []


def build_nc():
    nc = bacc.Bacc("TRN2", target_bir_lowering=False, debug=False, num_devices=N_CORES)

    xt_d = nc.dram_tensor("xt", [B, E, S], BF16, kind="ExternalInput").ap()
    wq_d = nc.dram_tensor("wq", [E, HPC * DH], BF16, kind="ExternalInput").ap()
    wk_d = nc.dram_tensor("wk", [E, HPC * DH], BF16, kind="ExternalInput").ap()
    wv_d = nc.dram_tensor("wv", [E, HPC * DH], BF16, kind="ExternalInput").ap()
    wo_d = nc.dram_tensor("wo", [E, E], BF16, kind="ExternalInput").ap()
    out_d = nc.dram_tensor("out", [B, QS, E], F32, kind="ExternalOutput").ap()

    with tile.TileContext(nc, trace_sim=False) as tc:
        with (
            tc.tile_pool(name="const", bufs=1) as constp,
            tc.tile_pool(name="wpool", bufs=1) as wpool,
            tc.tile_pool(name="wop", bufs=1) as wop,
            tc.tile_pool(name="xep", bufs=2) as xep,
            tc.tile_pool(name="qkv", bufs=2) as qkvp,
            tc.tile_pool(name="vst", bufs=2) as vstp,
            tc.tile_pool(name="pt", bufs=3) as ptp,
            tc.tile_pool(name="ct", bufs=2) as ctp,
            tc.tile_pool(name="norm", bufs=2) as normp,
            tc.tile_pool(name="cg", bufs=1) as cgp,
            tc.tile_pool(name="osb", bufs=2) as osbp,
            tc.tile_pool(name="psb", bufs=2, space="PSUM") as psb,  # [128,1024] x2 = 4 banks
            tc.tile_pool(name="psm", bufs=2, space="PSUM") as psm,  # [128,512] x2 = 2 banks
            tc.tile_pool(name="pav", bufs=2, space="PSUM") as pav,  # [65,512] x2 = 2 banks
            tc.tile_pool(name="dram", bufs=2, space="DRAM") as dramp,
        ):
            identb = constp.tile([128, 128], BF16, tag="identb")
            make_identity(nc, identb[:])
            ones_col = constp.tile([128, 1], BF16, tag="ones_col")
            nc.gpsimd.memset(ones_col[:], 1.0)
            # causal diagonal mask: 0 where q_rel >= t_rel else -8000
            # (accumulated into the diagonal score block BY THE PE, via
            # identity.T @ mtri -- keeps every other engine queue free of
            # mask dependencies; -8000 is exact in bf16)
            mtri = constp.tile([128, 128], BF16, tag="mtri")
            nc.gpsimd.memset(mtri[:], 0.0)
            nc.gpsimd.affine_select(
                out=mtri[:], in_=mtri[:],
                compare_op=mybir.AluOpType.is_ge, fill=-8000.0,
                base=0, pattern=[[1, 128]], channel_multiplier=-1,
            )

            # host pre-packs weights as [p, chunk, col] so each tensor is one
            # contiguous DMA; stationary tiles are slices of the big tile.
            # wq first, then batch-0 x^T chunks, so the first projection's
            # inputs land as early as possible.
            wall = {}
            wall["q"] = wpool.tile([128, 8 * 128], BF16, tag="wq_all", name="wq_all")
            nc.sync.dma_start(
                out=wall["q"][:].rearrange("p (c d) -> p c d", c=8),
                in_=wq_d[:].rearrange("(c p) d -> p c d", p=128),
            )

            # x^T tiles for both batches, prefetched up front on the sync
            # HWDGE queue: xe[b][ec] = x^T[e-chunk ec, all 2048 q] bf16
            xe = {}
            for ec in range(8):
                t = xep.tile([128, S], BF16, tag=f"xe{ec}", name=f"xe0_{ec}")
                nc.sync.dma_start(out=t[:], in_=xt_d[0, ec * 128:(ec + 1) * 128, :])
                xe[0, ec] = t

            for name, wd in (("k", wk_d), ("v", wv_d)):
                wall[name] = wpool.tile([128, 8 * 128], BF16, tag=f"w{name}_all", name=f"w{name}_all")
                nc.sync.dma_start(
                    out=wall[name][:].rearrange("p (c d) -> p c d", c=8),
                    in_=wd[:].rearrange("(c p) d -> p c d", p=128),
                )

            def wtiles(name, ec):
                return wall[name][:, ec * 128:(ec + 1) * 128]

            wo_all = wop.tile([128, 8 * 1024], BF16, tag="wo_all")
            nc.sync.dma_start(
                out=wo_all[:].rearrange("p (c d) -> p c d", c=8),
                in_=wo_d[:].rearrange("(c p) d -> p c d", p=128),
            )

            for ec in range(8):
                t = xep.tile([128, S], BF16, tag=f"xe{ec}", name=f"xe1_{ec}")
                nc.sync.dma_start(out=t[:], in_=xt_d[1, ec * 128:(ec + 1) * 128, :])
                xe[1, ec] = t

            copy_flip = [0]

            def copy_balanced(dst, src):
                # alternate psum->sbuf evictions between DVE and ACT
                if copy_flip[0] % 2 == 0:
                    nc.vector.tensor_copy(dst, src)
                else:
                    nc.scalar.copy(dst, src)
                copy_flip[0] += 1

            vst_init = {}

            def emit_batch(b):
                # ---- Phase A/B: QKV projections ---------------------------
                qkv = {}
                for name in ("q", "k", "v"):
                    for h in range(HPC):
                        qkv[name, h] = qkvp.tile([64, S], BF16, tag=f"{name}h{h}", name=f"{name}h{h}")
                for qg in range(4):
                    sl = slice(qg * 512, qg * 512 + 512)
                    for name in ("q", "k", "v"):
                        ps = psm.tile([128, 512], F32, tag="mm")
                        for ec in range(8):
                            nc.tensor.matmul(
                                ps[:],
                                wtiles(name, ec),
                                xe[b, ec][:, sl],
                                start=(ec == 0),
                                stop=(ec == 7),
                            )
                        nc.vector.tensor_copy(qkv[name, 0][:, sl], ps[0:64, :])
                        nc.scalar.copy(qkv[name, 1][:, sl], ps[64:128, :])

                # ---- Phase C: attention per head ---------------------------
                ct = [ctp.tile([64, S], BF16, tag=f"ct{h}", name=f"ct{h}") for h in range(HPC)]
                # one AllToAll per batch: shard j carries BOTH heads' C^T
                # columns for core j ([128, QS] stacked) -> half the sync
                # points vs per-head collectives
                a2a_in = dramp.tile([8, 2 * DH, QS], BF16, tag="a2a_in", name=f"a2a_in{b}")
                a2a_out = dramp.tile([8, 2 * DH, QS], BF16, tag="a2a_out", name=f"a2a_out{b}")
                for h in range(HPC):
                    vh = qkv["v", h]
                    # Vst[tc]: [128 t, 65] = [V_chunk | ones]
                    vst = []
                    for tg in range(2):  # pack 8 transposes per psum tile
                        ps = psm.tile([128, 512], BF16, tag="mm")
                        for k in range(8):
                            tcx = tg * 8 + k
                            nc.tensor.transpose(
                                ps[:, k * 64:(k + 1) * 64],
                                vh[:, tcx * 128:(tcx + 1) * 128],
                                identb[0:64, 0:64],
                            )
                        for k in range(8):
                            tcx = tg * 8 + k
                            vt = vstp.tile(
                                [128, 65], BF16, tag=f"vst{tcx}",
                                name=f"vst{tcx}",
                            )
                            nc.vector.tensor_copy(vt[:, 0:64], ps[:, k * 64:(k + 1) * 64])
                            # ones column persists in the slot; write it only
                            # the first time each slot comes around
                            cnt = vst_init.get(tcx, 0)
                            if cnt < 2:
                                nc.vector.tensor_copy(vt[:, 64:65], ones_col[:])
                                vst_init[tcx] = cnt + 1
                            vst.append(vt)

                    kh, qh = qkv["k", h], qkv["q", h]
                    for half in range(2):
                        qbase = half * 1024
                        n_tc = 8 * (half + 1)
                        av = [
                            pav.tile([65, 512], F32, tag="av", name=f"av{i}")
                            for i in range(2)
                        ]
                        for tcx in range(n_tc):
                            t0 = tcx * 128
                            q_lo = max(t0, qbase)
                            lo_rel = q_lo - qbase
                            strip = psb.tile([128, 1024], F32, tag="big")
                            # scores into strip (columns relative to qbase);
                            # ISA caps matmul output at 512 fp32 cols. The
                            # causal diagonal block (always inside the first
                            # seg) gets -8000 accumulated by a second PE
                            # matmul (identity stationary, mtri moving).
                            diag = t0 >= qbase
                            segs = []
                            if lo_rel < 512:
                                segs.append((lo_rel, 512))
                                segs.append((512, 1024))
                            else:
                                segs.append((lo_rel, 1024))
                            for s0, s1 in segs:
                                first = s0 == lo_rel
                                nc.tensor.matmul(
                                    strip[:, s0:s1],
                                    kh[:, t0:t0 + 128],
                                    qh[:, qbase + s0:qbase + s1],
                                    start=True,
                                    stop=not (diag and first),
                                )
                                if diag and first:
                                    nc.tensor.matmul(
                                        strip[:, lo_rel:lo_rel + 128],
                                        identb[:],
                                        mtri[:],
                                        start=False,
                                        stop=True,
                                    )
                            pt = ptp.tile([128, 1024], BF16, tag="pt")
                            nc.scalar.activation(
                                pt[:, lo_rel:1024],
                                strip[:, lo_rel:1024],
                                AF.Exp,
                                scale=SCALE,
                            )
                            for qbr in range(2):
                                qb = 2 * half + qbr
                                if qb * 512 + 512 <= t0:
                                    continue
                                m_lo = max(t0, qb * 512)
                                nc.tensor.matmul(
                                    av[qbr][:, m_lo - qb * 512:512],
                                    vst[tcx][:],
                                    pt[:, m_lo - qbase:qb * 512 + 512 - qbase],
                                    start=(tcx == 0),
                                    stop=(tcx == (qb + 1) * 4 - 1),
                                )
                        # normalize the two q-blocks of this half
                        for qbr in range(2):
                            qb = 2 * half + qbr
                            # denominator row to partition 0 (ACT can shift)
                            nsb = normp.tile([1, 512], F32, tag="nsb")
                            nc.scalar.copy(nsb[:], av[qbr][64:65, :])
                            recip1 = normp.tile([1, 512], F32, tag="recip1")
                            nc.vector.reciprocal_approx_fast(recip1[:], nsb[:])
                            bc = normp.tile([64, 512], F32, tag="bc")
                            nc.gpsimd.partition_broadcast(bc[:], recip1[:])
                            nc.vector.tensor_mul(
                                ct[h][:, qb * 512:qb * 512 + 512],
                                av[qbr][0:64, :],
                                bc[:],
                            )
                            # ship this q-block's two a2a shards immediately
                            for j in (2 * qb, 2 * qb + 1):
                                nc.gpsimd.dma_start(
                                    out=a2a_in[j, h * DH:(h + 1) * DH, :],
                                    in_=ct[h][:, j * QS:(j + 1) * QS],
                                )

                nc.gpsimd.collective_compute(
                    "AllToAll",
                    mybir.AluOpType.bypass,
                    replica_groups=[list(range(N_CORES))],
                    ins=[a2a_in[:].opt()],
                    outs=[a2a_out[:].opt()],
                )

                return a2a_out, ct

            def emit_e(b, cgbig):
                # ---- Phase E: output projection ---------------------------
                for qt in range(QS // 128):
                    osb = osbp.tile([128, 1024], F32, tag="osb")
                    for oh in range(2):
                        ps = psm.tile([128, 512], F32, tag="mm")
                        for i in range(8):
                            nc.tensor.matmul(
                                ps[:],
                                cgbig[:, i * QS + qt * 128:i * QS + (qt + 1) * 128],
                                wo_all[:, i * 1024 + oh * 512:i * 1024 + (oh + 1) * 512],
                                start=(i == 0),
                                stop=(i == 7),
                            )
                        copy_balanced(osb[:, oh * 512:(oh + 1) * 512], ps[:])
                    nc.sync.dma_start(
                        out=out_d[b, qt * 128:(qt + 1) * 128, :], in_=osb[:]
                    )

            # Both E phases are emitted LAST. The Tile scheduler models
            # collectives as cheap and would otherwise hoist E(0) (which
            # depends on batch 0's ~30us AllToAll) ahead of batch-1 work in
            # the in-order PE queue, stalling the machine mid-kernel. A
            # 1-element dummy write into the cg tile, sourced from batch 1's
            # final attention output, makes the gather (whole-tile WAW) and
            # hence E(0) unschedulable before batch 1 finishes -- E(0) then
            # fills the final collective's flight time.
            outs = [emit_batch(b) for b in range(B)]
            with tc.high_priority(offset=-1000000):
                cg0 = cgp.tile([128, 8 * QS], BF16, tag="cgbig", name="cg0", bufs=2)
                nc.vector.tensor_copy(cg0[0:1, 0:1], outs[1][1][1][0:1, 0:1])
                nc.sync.dma_start(
                    out=cg0[:].rearrange("p (i q) -> p i q", i=8),
                    in_=outs[0][0][:].rearrange("i p q -> p i q"),
                )
                emit_e(0, cg0)
                cg1 = cgp.tile([128, 8 * QS], BF16, tag="cgbig", name="cg1", bufs=2)
                nc.sync.dma_start(
                    out=cg1[:].rearrange("p (i q) -> p i q", i=8),
                    in_=outs[1][0][:].rearrange("i p q -> p i q"),
                )
                emit_e(1, cg1)

    nc.compile()
    return nc


def _get_nc():
    if not _NC_CACHE:
        _NC_CACHE.append(build_nc())
    return _NC_CACHE[0]


def run(inputs, trace=False, trace_cores=None):
    nc = _get_nc()
    x = np.asarray(inputs["x"], np.float32)
    xt = np.ascontiguousarray(x.transpose(0, 2, 1)).astype(bfloat16)
    Wq = np.asarray(inputs["Wq"], np.float32)
    Wk = np.asarray(inputs["Wk"], np.float32)
    Wv = np.asarray(inputs["Wv"], np.float32)
    W_O = np.ascontiguousarray(np.asarray(inputs["W_O"], np.float32)).astype(bfloat16)

    in_maps = []
    for j in range(N_CORES):
        h0 = HPC * j
        in_maps.append(
            {
                "xt": xt,
                "wq": np.ascontiguousarray(
                    np.concatenate([Wq[h0 + i] for i in range(HPC)], axis=1)
                ).astype(bfloat16),
                "wk": np.ascontiguousarray(
                    np.concatenate([Wk[h0 + i] for i in range(HPC)], axis=1)
                ).astype(bfloat16),
                "wv": np.ascontiguousarray(
                    np.concatenate([Wv[h0 + i] for i in range(HPC)], axis=1)
                ).astype(bfloat16),
                "wo": W_O,
            }
        )
    kwargs = {}
    if trace:
        kwargs["trace"] = True
        if trace_cores is not None:
            kwargs["trace_cores"] = trace_cores
    res = run_bass_kernel_spmd(nc, in_maps, core_ids=list(range(N_CORES)), **kwargs)
    out = np.empty((B, S, E), np.float32)
    for j in range(N_CORES):
        out[:, j * QS:(j + 1) * QS, :] = res.results[j]["out"]
    return out, res


def kernel(**inputs) -> np.ndarray:
    out, _ = run(inputs)
    return out


# revision 28
# speedup vs baseline: 1.0835x; 1.0168x over previous
"""Multi-head attention (B=2, S=2048, E=1024, H=16, d_h=64, causal) on 8
Trainium2 NeuronCores, bf16 matmuls (rel err ~5e-3, gate 2e-2).

Sharding: tensor-parallel over heads (2 heads/core) for QKV projections and
attention; AllToAll of the concatenated head outputs (1MB/core bf16); then
sequence-parallel output projection (each core computes 256 output rows per
batch).

v2 changes vs v1:
  - x is transposed + cast to bf16 on the host -> x^T tiles DMA straight
    into SBUF (HWDGE on sync), eliminating all PE transposes of x and their
    PSUM evictions, and the SWDGE cast bottleneck at kernel start.
  - all weights host-cast to bf16, HWDGE loads.
  - all 16 x^T tiles (both batches) prefetched at kernel start.
  - cg gathers moved to the sync queue so a slow collective cannot block
    the gpsimd queue (which attention's normalize needs) -> no cross-batch
    stall cascade.
  - normalize reads the AV accumulator straight from PSUM (drops the avsb
    eviction copy on ACT).
  - vst tiles keep a persistent ones-column (written once per slot).

Layouts (all "transposed", partition = contraction dim):
  x^T [e, s] from host; Q^T/K^T/V^T per head [64 d, 2048 s] from projection
  scores S^T [t, q] (stationary = K^T slice, moving = Q^T)
  exp on ACT; causal diagonal handled by adding a -8000 triangle pre-exp
  AV with stationary [V_chunk | ones] [128, 65] -> psum row 64 = softmax
  denominator; ACT shifts it to partition 0, DVE reciprocal, gpsimd
  partition_broadcast, DVE multiply normalizes into C^T
  W_O with stationary C^T chunks -> output directly in [q, e] layout.

Hardware constraints honored (empirical): matmul operands share
base_partition 0; DVE cannot shift partitions (ACT can); PSUM has no DMA
route.
"""

import numpy as np
from ml_dtypes import bfloat16

import concourse.bacc as bacc
import concourse.mybir as mybir
import concourse.tile as tile
from concourse.bass_utils import run_bass_kernel_spmd
from concourse.masks import make_identity

F32 = mybir.dt.float32
BF16 = mybir.dt.bfloat16
AF = mybir.ActivationFunctionType

N_CORES = 8
B, S, E = 2, 2048, 1024
H, DH = 16, 64
HPC = H // N_CORES  # heads per core = 2
QS = S // N_CORES  # output q rows per core per batch = 256
SCALE = 1.0 / 8.0  # 1/sqrt(DH)

_NC_CACHE = []


def build_nc():
    nc = bacc.Bacc("TRN2", target_bir_lowering=False, debug=False, num_devices=N_CORES)

    xt_d = nc.dram_tensor("xt", [B, E, S], BF16, kind="ExternalInput").ap()
    wq_d = nc.dram_tensor("wq", [E, HPC * DH], BF16, kind="ExternalInput").ap()
    wk_d = nc.dram_tensor("wk", [E, HPC * DH], BF16, kind="ExternalInput").ap()
    wv_d = nc.dram_tensor("wv", [E, HPC * DH], BF16, kind="ExternalInput").ap()
    wo_d = nc.dram_tensor("wo", [E, E], BF16, kind="ExternalInput").ap()
    out_d = nc.dram_tensor("out", [B, QS, E], F32, kind="ExternalOutput").ap()

    with tile.TileContext(nc, trace_sim=False) as tc:
        with (
            tc.tile_pool(name="const", bufs=1) as constp,
            tc.tile_pool(name="wpool", bufs=1) as wpool,
            tc.tile_pool(name="wop", bufs=1) as wop,
            tc.tile_pool(name="xep", bufs=2) as xep,
            tc.tile_pool(name="qkv", bufs=2) as qkvp,
            tc.tile_pool(name="vst", bufs=2) as vstp,
            tc.tile_pool(name="pt", bufs=3) as ptp,
            tc.tile_pool(name="ct", bufs=2) as ctp,
            tc.tile_pool(name="norm", bufs=2) as normp,
            tc.tile_pool(name="cg", bufs=1) as cgp,
            tc.tile_pool(name="osb", bufs=2) as osbp,
            tc.tile_pool(name="psb", bufs=2, space="PSUM") as psb,  # [128,1024] x2 = 4 banks
            tc.tile_pool(name="psm", bufs=2, space="PSUM") as psm,  # [128,512] x2 = 2 banks
            tc.tile_pool(name="pav", bufs=2, space="PSUM") as pav,  # [65,512] x2 = 2 banks
            tc.tile_pool(name="dram", bufs=2, space="DRAM") as dramp,
        ):
            identb = constp.tile([128, 128], BF16, tag="identb")
            make_identity(nc, identb[:])
            ones_col = constp.tile([128, 1], BF16, tag="ones_col")
            nc.gpsimd.memset(ones_col[:], 1.0)
            # causal diagonal mask: 0 where q_rel >= t_rel else -8000
            # (accumulated into the diagonal score block BY THE PE, via
            # identity.T @ mtri -- keeps every other engine queue free of
            # mask dependencies; -8000 is exact in bf16)
            mtri = constp.tile([128, 128], BF16, tag="mtri")
            nc.gpsimd.memset(mtri[:], 0.0)
            nc.gpsimd.affine_select(
                out=mtri[:], in_=mtri[:],
                compare_op=mybir.AluOpType.is_ge, fill=-8000.0,
                base=0, pattern=[[1, 128]], channel_multiplier=-1,
            )

            # host pre-packs weights as [p, chunk, col] so each tensor is one
            # contiguous DMA; stationary tiles are slices of the big tile.
            # wq first, then batch-0 x^T chunks, so the first projection's
            # inputs land as early as possible.
            # weights go on the scalar HWDGE queue, x^T chunks alternate
            # between sync and scalar: two DMA rings land the inputs ~2x
            # faster, which is what gates the first projections
            wall = {}
            wall["q"] = wpool.tile([128, 8 * 128], BF16, tag="wq_all", name="wq_all")
            nc.scalar.dma_start(
                out=wall["q"][:].rearrange("p (c d) -> p c d", c=8),
                in_=wq_d[:].rearrange("(c p) d -> p c d", p=128),
            )

            xe = {}
            for b in range(B):
                for ec in range(8):
                    t = xep.tile([128, S], BF16, tag=f"xe{ec}", name=f"xe{b}_{ec}")
                    eng = nc.sync if ec % 2 == 0 else nc.scalar
                    eng.dma_start(out=t[:], in_=xt_d[b, ec * 128:(ec + 1) * 128, :])
                    xe[b, ec] = t
                if b == 0:
                    for name, wd in (("k", wk_d), ("v", wv_d)):
                        wall[name] = wpool.tile([128, 8 * 128], BF16, tag=f"w{name}_all", name=f"w{name}_all")
                        nc.scalar.dma_start(
                            out=wall[name][:].rearrange("p (c d) -> p c d", c=8),
                            in_=wd[:].rearrange("(c p) d -> p c d", p=128),
                        )

            def wtiles(name, ec):
                return wall[name][:, ec * 128:(ec + 1) * 128]

            wo_all = wop.tile([128, 8 * 1024], BF16, tag="wo_all")
            nc.scalar.dma_start(
                out=wo_all[:].rearrange("p (c d) -> p c d", c=8),
                in_=wo_d[:].rearrange("(c p) d -> p c d", p=128),
            )

            copy_flip = [0]

            def copy_balanced(dst, src):
                # alternate psum->sbuf evictions between DVE and ACT
                if copy_flip[0] % 2 == 0:
                    nc.vector.tensor_copy(dst, src)
                else:
                    nc.scalar.copy(dst, src)
                copy_flip[0] += 1

            vst_init = {}

            def emit_batch(b):
                # ---- Phase A/B: QKV projections ---------------------------
                qkv = {}
                for name in ("q", "k", "v"):
                    for h in range(HPC):
                        qkv[name, h] = qkvp.tile([64, S], BF16, tag=f"{name}h{h}", name=f"{name}h{h}")
                for qg in range(4):
                    sl = slice(qg * 512, qg * 512 + 512)
                    for name in ("q", "k", "v"):
                        ps = psm.tile([128, 512], F32, tag="mm")
                        for ec in range(8):
                            nc.tensor.matmul(
                                ps[:],
                                wtiles(name, ec),
                                xe[b, ec][:, sl],
                                start=(ec == 0),
                                stop=(ec == 7),
                            )
                        nc.vector.tensor_copy(qkv[name, 0][:, sl], ps[0:64, :])
                        nc.scalar.copy(qkv[name, 1][:, sl], ps[64:128, :])

                # ---- Phase C: attention per head ---------------------------
                ct = [ctp.tile([64, S], BF16, tag=f"ct{h}", name=f"ct{h}") for h in range(HPC)]
                # one AllToAll per batch: shard j carries BOTH heads' C^T
                # columns for core j ([128, QS] stacked) -> half the sync
                # points vs per-head collectives
                a2a_in = dramp.tile([8, 2 * DH, QS], BF16, tag="a2a_in", name=f"a2a_in{b}")
                a2a_out = dramp.tile([8, 2 * DH, QS], BF16, tag="a2a_out", name=f"a2a_out{b}")
                for h in range(HPC):
                    vh = qkv["v", h]
                    # Vst[tc]: [128 t, 65] = [V_chunk | ones]
                    vst = []
                    for tg in range(2):  # pack 8 transposes per psum tile
                        ps = psm.tile([128, 512], BF16, tag="mm")
                        for k in range(8):
                            tcx = tg * 8 + k
                            nc.tensor.transpose(
                                ps[:, k * 64:(k + 1) * 64],
                                vh[:, tcx * 128:(tcx + 1) * 128],
                                identb[0:64, 0:64],
                            )
                        for k in range(8):
                            tcx = tg * 8 + k
                            vt = vstp.tile(
                                [128, 65], BF16, tag=f"vst{tcx}",
                                name=f"vst{tcx}",
                            )
                            nc.vector.tensor_copy(vt[:, 0:64], ps[:, k * 64:(k + 1) * 64])
                            # ones column persists in the slot; write it only
                            # the first time each slot comes around
                            cnt = vst_init.get(tcx, 0)
                            if cnt < 2:
                                nc.vector.tensor_copy(vt[:, 64:65], ones_col[:])
                                vst_init[tcx] = cnt + 1
                            vst.append(vt)

                    kh, qh = qkv["k", h], qkv["q", h]
                    for half in range(2):
                        qbase = half * 1024
                        n_tc = 8 * (half + 1)
                        av = [
                            pav.tile([65, 512], F32, tag="av", name=f"av{i}")
                            for i in range(2)
                        ]
                        for tcx in range(n_tc):
                            t0 = tcx * 128
                            q_lo = max(t0, qbase)
                            lo_rel = q_lo - qbase
                            strip = psb.tile([128, 1024], F32, tag="big")
                            # scores into strip (columns relative to qbase);
                            # ISA caps matmul output at 512 fp32 cols. The
                            # causal diagonal block (always inside the first
                            # seg) gets -8000 accumulated by a second PE
                            # matmul (identity stationary, mtri moving).
                            diag = t0 >= qbase
                            segs = []
                            if lo_rel < 512:
                                segs.append((lo_rel, 512))
                                segs.append((512, 1024))
                            else:
                                segs.append((lo_rel, 1024))
                            for s0, s1 in segs:
                                first = s0 == lo_rel
                                nc.tensor.matmul(
                                    strip[:, s0:s1],
                                    kh[:, t0:t0 + 128],
                                    qh[:, qbase + s0:qbase + s1],
                                    start=True,
                                    stop=not (diag and first),
                                )
                                if diag and first:
                                    nc.tensor.matmul(
                                        strip[:, lo_rel:lo_rel + 128],
                                        identb[:],
                                        mtri[:],
                                        start=False,
                                        stop=True,
                                    )
                            pt = ptp.tile([128, 1024], BF16, tag="pt")
                            nc.scalar.activation(
                                pt[:, lo_rel:1024],
                                strip[:, lo_rel:1024],
                                AF.Exp,
                                scale=SCALE,
                            )
                            for qbr in range(2):
                                qb = 2 * half + qbr
                                if qb * 512 + 512 <= t0:
                                    continue
                                m_lo = max(t0, qb * 512)
                                nc.tensor.matmul(
                                    av[qbr][:, m_lo - qb * 512:512],
                                    vst[tcx][:],
                                    pt[:, m_lo - qbase:qb * 512 + 512 - qbase],
                                    start=(tcx == 0),
                                    stop=(tcx == (qb + 1) * 4 - 1),
                                )
                        # normalize the two q-blocks of this half
                        for qbr in range(2):
                            qb = 2 * half + qbr
                            # denominator row to partition 0 (ACT can shift)
                            nsb = normp.tile([1, 512], F32, tag="nsb")
                            nc.scalar.copy(nsb[:], av[qbr][64:65, :])
                            recip1 = normp.tile([1, 512], F32, tag="recip1")
                            nc.vector.reciprocal_approx_fast(recip1[:], nsb[:])
                            bc = normp.tile([64, 512], F32, tag="bc")
                            nc.gpsimd.partition_broadcast(bc[:], recip1[:])
                            nc.vector.tensor_mul(
                                ct[h][:, qb * 512:qb * 512 + 512],
                                av[qbr][0:64, :],
                                bc[:],
                            )
                            # ship this q-block's two a2a shards immediately
                            for j in (2 * qb, 2 * qb + 1):
                                nc.gpsimd.dma_start(
                                    out=a2a_in[j, h * DH:(h + 1) * DH, :],
                                    in_=ct[h][:, j * QS:(j + 1) * QS],
                                )

                nc.gpsimd.collective_compute(
                    "AllToAll",
                    mybir.AluOpType.bypass,
                    replica_groups=[list(range(N_CORES))],
                    ins=[a2a_in[:].opt()],
                    outs=[a2a_out[:].opt()],
                )

                return a2a_out, ct

            def emit_e(b, cgbig):
                # ---- Phase E: output projection ---------------------------
                for qt in range(QS // 128):
                    osb = osbp.tile([128, 1024], F32, tag="osb")
                    for oh in range(2):
                        ps = psm.tile([128, 512], F32, tag="mm")
                        for i in range(8):
                            nc.tensor.matmul(
                                ps[:],
                                cgbig[:, i * QS + qt * 128:i * QS + (qt + 1) * 128],
                                wo_all[:, i * 1024 + oh * 512:i * 1024 + (oh + 1) * 512],
                                start=(i == 0),
                                stop=(i == 7),
                            )
                        copy_balanced(osb[:, oh * 512:(oh + 1) * 512], ps[:])
                    nc.sync.dma_start(
                        out=out_d[b, qt * 128:(qt + 1) * 128, :], in_=osb[:]
                    )

            # Both E phases are emitted LAST. The Tile scheduler models
            # collectives as cheap and would otherwise hoist E(0) (which
            # depends on batch 0's ~30us AllToAll) ahead of batch-1 work in
            # the in-order PE queue, stalling the machine mid-kernel. A
            # 1-element dummy write into the cg tile, sourced from batch 1's
            # final attention output, makes the gather (whole-tile WAW) and
            # hence E(0) unschedulable before batch 1 finishes -- E(0) then
            # fills the final collective's flight time.
            outs = [emit_batch(b) for b in range(B)]
            with tc.high_priority(offset=-1000000):
                cg0 = cgp.tile([128, 8 * QS], BF16, tag="cgbig", name="cg0", bufs=2)
                nc.vector.tensor_copy(cg0[0:1, 0:1], outs[1][1][1][0:1, 0:1])
                nc.sync.dma_start(
                    out=cg0[:].rearrange("p (i q) -> p i q", i=8),
                    in_=outs[0][0][:].rearrange("i p q -> p i q"),
                )
                emit_e(0, cg0)
                cg1 = cgp.tile([128, 8 * QS], BF16, tag="cgbig", name="cg1", bufs=2)
                nc.sync.dma_start(
                    out=cg1[:].rearrange("p (i q) -> p i q", i=8),
                    in_=outs[1][0][:].rearrange("i p q -> p i q"),
                )
                emit_e(1, cg1)

    nc.compile()
    return nc


def _get_nc():
    if not _NC_CACHE:
        _NC_CACHE.append(build_nc())
    return _NC_CACHE[0]


def run(inputs, trace=False, trace_cores=None):
    nc = _get_nc()
    x = np.asarray(inputs["x"], np.float32)
    xt = np.ascontiguousarray(x.transpose(0, 2, 1)).astype(bfloat16)
    Wq = np.asarray(inputs["Wq"], np.float32)
    Wk = np.asarray(inputs["Wk"], np.float32)
    Wv = np.asarray(inputs["Wv"], np.float32)
    W_O = np.ascontiguousarray(np.asarray(inputs["W_O"], np.float32)).astype(bfloat16)

    in_maps = []
    for j in range(N_CORES):
        h0 = HPC * j
        in_maps.append(
            {
                "xt": xt,
                "wq": np.ascontiguousarray(
                    np.concatenate([Wq[h0 + i] for i in range(HPC)], axis=1)
                ).astype(bfloat16),
                "wk": np.ascontiguousarray(
                    np.concatenate([Wk[h0 + i] for i in range(HPC)], axis=1)
                ).astype(bfloat16),
                "wv": np.ascontiguousarray(
                    np.concatenate([Wv[h0 + i] for i in range(HPC)], axis=1)
                ).astype(bfloat16),
                "wo": W_O,
            }
        )
    kwargs = {}
    if trace:
        kwargs["trace"] = True
        if trace_cores is not None:
            kwargs["trace_cores"] = trace_cores
    res = run_bass_kernel_spmd(nc, in_maps, core_ids=list(range(N_CORES)), **kwargs)
    out = np.empty((B, S, E), np.float32)
    for j in range(N_CORES):
        out[:, j * QS:(j + 1) * QS, :] = res.results[j]["out"]
    return out, res


def kernel(**inputs) -> np.ndarray:
    out, _ = run(inputs)
    return out
